# revision 13
# baseline (speedup 1.0000x reference)
"""AttentiveGraphConvolution (GAT-style layer) on 8 trn2 NeuronCores.

Math (reference):
    h   = x @ W                       [N, D]
    a_s = h @ attn_self               [N, 1]
    a_n = h @ attn_neigh              [N, 1]
    e   = leaky_relu(a_s + a_n.T, 0.2)
    e   = e + NEG_INF * (1 - adj)
    out = relu(softmax(e, -1) @ h)

Reformulation (exact in fp32 up to rounding; leaky alpha = 0.2):
    t_ij = adj_ij * exp(leaky(a_s_i + a_n_j))
         = adj_ij * u2_i * v2_j * max(w_i * w2_j, 1)
    with u2 = e^{0.2 a_s}, v2 = e^{0.2 a_n}, w = e^{0.8 a_s}, w2 = e^{0.8 a_n}.
    u2_i cancels in the softmax ratio, so with
        m_j   = e^{-0.8 a_n_j}
        q_ji  = adjT_ji * max(w_i, m_j)          (one DVE op per tile)
        h2_j  = e^{a_n_j} * h_j                  (scaled once, pre-gather)
        ean_j = e^{a_n_j}
    out_i = relu( (sum_j q_ji h2_j) / (sum_j q_ji ean_j) ).

Per adj chunk [128 j, 1024 i] the hot loop is exactly:
    DVE  scalar_tensor_tensor: q = (wb max m_j) * adjT      (all bf16, 2x mode)
    PE   matmul: outT[d, i]  += h2_chunk[j, d].T @ q        (bf16)
    PE   matmul: rs[1, i]    += ean_chunk[j, 1].T @ q       (bf16)
No scalar-engine work per chunk; wb (= w_i broadcast) is a fixed SBUF tile.

Scheduling: a_s/a_n are computed directly from x @ (W @ att) so the tiny
a_n AllGather triggers within ~5 us; the h2 AllGather (bf16, 2 MB) follows
immediately after the local h shard is built.  The whole adj row-slab
(16.8 MB bf16) is prefetched into SBUF while the cores rendezvous at the
collectives, so the post-gather main loop is engine-bound, not DMA-bound.

Sharding: output rows across 8 cores; adj arrives pre-transposed bf16 with
a 4-row interleave for 8 KB DMA descriptors (host layout prep only).
"""

import numpy as np

N = 8192
DIN = 512
DOUT = 128
NCORES = 8
S = N // NCORES     # 1024 output rows per core
GP = 4              # adjacency rows per partition per DMA (descriptor size)


def _emit(nc, tc, ctx, n, s, din, dout):
    from concourse import masks, mybir

    f32 = mybir.dt.float32
    bf16 = mybir.dt.bfloat16
    AF = mybir.ActivationFunctionType
    ALU = mybir.AluOpType

    P = 128
    jc_n = n // P       # j chunks over all nodes (64)
    sc_n = s // P       # chunks in the local row slab (8)
    kc_n = din // P     # contraction chunks for x @ W (4)
    g_n = jc_n // GP    # adj super-chunks (GP j-chunks per DMA) (16)

    adjt = nc.dram_tensor("adjt", [n, s], bf16, kind="ExternalInput")
    xt = nc.dram_tensor("xt", [din, s], bf16, kind="ExternalInput")
    wmat = nc.dram_tensor("wmat", [din, dout], bf16, kind="ExternalInput")
    wt = nc.dram_tensor("wt", [dout, din], bf16, kind="ExternalInput")
    att = nc.dram_tensor("att", [dout, 2], bf16, kind="ExternalInput")
    out = nc.dram_tensor("out", [s, dout], f32, kind="ExternalOutput")

    const_pool = ctx.enter_context(tc.tile_pool(name="const", bufs=1))
    ph1_pool = ctx.enter_context(tc.tile_pool(name="ph1", bufs=1))
    tp_psum = ctx.enter_context(tc.tile_pool(name="tp_psum", bufs=2, space="PSUM"))
    acc_psum = ctx.enter_context(tc.tile_pool(name="acc_psum", bufs=1, space="PSUM"))
    dram_pool = ctx.enter_context(tc.tile_pool(name="dram", bufs=1, space="DRAM"))
    adj_pool = ctx.enter_context(tc.tile_pool(name="adj", bufs=1))
    q_pool = ctx.enter_context(tc.tile_pool(name="q", bufs=8))
    fin_pool = ctx.enter_context(tc.tile_pool(name="fin", bufs=2))

    groups = [list(range(NCORES))]

    # ---- Phase 0: input DMAs (small tensors first, then the adj stream) ----
    w_sb = []
    x_sb = []
    for k in range(kc_n):
        wk = ph1_pool.tile([P, dout], bf16, name="w_sb", tag=f"w_sb{k}")
        nc.sync.dma_start(wk[:], wmat[k * P:(k + 1) * P, :])
        w_sb.append(wk)
    wt_sb = ph1_pool.tile([P, din], bf16, name="wt_sb")
    nc.sync.dma_start(wt_sb[:], wt[:])
    att_sb = const_pool.tile([P, 2], bf16, name="att_sb")
    nc.sync.dma_start(att_sb[:], att[:])
    for k in range(kc_n):
        xk = ph1_pool.tile([P, s], bf16, name="x_sb", tag=f"x_sb{k}")
        nc.sync.dma_start(xk[:], xt[k * P:(k + 1) * P, :])
        x_sb.append(xk)

    adj_t = []
    for g in range(g_n):
        at = adj_pool.tile([P, GP * s], bf16, name="adj_t", tag=f"adj{g}")
        nc.sync.dma_start(
            at[:],
            adjt[g * GP * P:(g + 1) * GP * P, :].rearrange(
                "(p r) i -> p (r i)", r=GP),
        )
        adj_t.append(at)

    ident = const_pool.tile([P, P], f32, name="ident")
    masks.make_identity(nc, ident[:])
    identb = const_pool.tile([P, P], bf16, name="identb")
    nc.scalar.activation(identb[:], ident[:], AF.Copy)
    ones_sb = const_pool.tile([1, P], f32, name="ones_sb")
    nc.gpsimd.memset(ones_sb[:], 1.0)

    # ---- Phase 1a: attention logit vectors straight from x ---------------
    # av2 = [Ws | Wn] = attT @ WT : [2, din]
    av2_ps = tp_psum.tile([2, din], f32, name="av2_ps", tag="tp")
    nc.tensor.matmul(av2_ps[:], att_sb[:], wt_sb[:], start=True, stop=True)
    av2_sb = ph1_pool.tile([2, din], bf16, name="av2_sb")
    nc.scalar.activation(av2_sb[:], av2_ps[:], AF.Copy)
    av2T_sb = []
    for k in range(kc_n):
        avT_ps = tp_psum.tile([P, 2], bf16, name="avT_ps", tag="tp")
        nc.tensor.matmul(
            avT_ps[:], av2_sb[:, k * P:(k + 1) * P], identb[:2, :2],
            is_transpose=True, start=True, stop=True,
        )
        a2t = ph1_pool.tile([P, 2], bf16, name="av2T_sb", tag=f"av2T{k}")
        nc.scalar.activation(a2t[:], avT_ps[:], AF.Copy)
        av2T_sb.append(a2t)
    # av[2, s] = [a_s ; a_n] for the local slab
    av_sb = ph1_pool.tile([2, s], f32, name="av_sb")
    for b in range(s // 512):
        av_ps = tp_psum.tile([2, 512], f32, name="av_ps", tag="tp")
        for k in range(kc_n):
            nc.tensor.matmul(
                av_ps[:], av2T_sb[k][:], x_sb[k][:, b * 512:(b + 1) * 512],
                start=(k == 0), stop=(k == kc_n - 1),
            )
        nc.scalar.activation(av_sb[:, b * 512:(b + 1) * 512], av_ps[:], AF.Copy)

    # ---- Phase 1b: a_n AllGather (tiny; gates all remote q scalars) ------
    an_dram = dram_pool.tile([s, 1], f32, name="an_dram")
    nc.sync.dma_start(an_dram[:].rearrange("s o -> o s"), av_sb[1:2, :])
    anfull_dram = dram_pool.tile([n, 1], f32, addr_space="Shared", name="anfull")
    nc.gpsimd.collective_compute(
        "AllGather", ALU.bypass, replica_groups=groups,
        ins=[an_dram.opt()], outs=[anfull_dram.opt()],
    )

    # ---- Phase 1c: local h shard, scaled h2 = e^{a_n} h, pushed to DRAM --
    hT_sb = ph1_pool.tile([P, s], bf16, name="hT_sb")
    for b in range(s // 512):
        hT_ps = tp_psum.tile([P, 512], f32, name="hT_ps", tag="tp")
        for k in range(kc_n):
            nc.tensor.matmul(
                hT_ps[:], w_sb[k][:], x_sb[k][:, b * 512:(b + 1) * 512],
                start=(k == 0), stop=(k == kc_n - 1),
            )
        nc.scalar.activation(hT_sb[:, b * 512:(b + 1) * 512], hT_ps[:], AF.Copy)

    # local a_n in [node-partition, chunk] layout -> e^{a_n} scale vector
    anT_sb = ph1_pool.tile([P, sc_n], f32, name="anT_sb")
    for c in range(sc_n):
        avT2_ps = tp_psum.tile([P, 2], f32, name="avT2_ps", tag="tp")
        nc.tensor.matmul(
            avT2_ps[:], av_sb[:, c * P:(c + 1) * P], ident[:2, :2],
            is_transpose=True, start=True, stop=True,
        )
        nc.scalar.activation(anT_sb[:, c:c + 1], avT2_ps[:, 1:2], AF.Copy)
    eanloc_sb = ph1_pool.tile([P, sc_n], f32, name="eanloc_sb")
    nc.scalar.activation(eanloc_sb[:], anT_sb[:], AF.Exp, scale=1.0)

    # h2 local chunks written partition-major for 2 KB read-back descriptors
    h2_dram = dram_pool.tile([s, dout], bf16, name="h2_dram")
    h2_pm = h2_dram[:].rearrange("(p kl) d -> kl p d", kl=sc_n)
    for c in range(sc_n):
        hn_ps = tp_psum.tile([P, P], bf16, name="hn_ps", tag="tp")
        nc.tensor.matmul(
            hn_ps[:], hT_sb[:, c * P:(c + 1) * P], identb[:],
            is_transpose=True, start=True, stop=True,
        )
        h2c_sb = fin_pool.tile([P, dout], bf16, name="h2c_sb")
        nc.scalar.activation(h2c_sb[:], hn_ps[:], AF.Copy,
                             scale=eanloc_sb[:, c:c + 1])
        nc.sync.dma_start(h2_pm[c], h2c_sb[:])

    h2full_dram = dram_pool.tile([n, dout], bf16, addr_space="Shared",
                                 name="h2full")
    nc.gpsimd.collective_compute(
        "AllGather", ALU.bypass, replica_groups=groups,
        ins=[h2_dram.opt()], outs=[h2full_dram.opt()],
    )

    # ---- Phase 2: wb = e^{0.8 a_s_i} broadcast tile (fixed, bf16) --------
    wrow_sb = ph1_pool.tile([1, s], f32, name="wrow_sb")
    nc.scalar.activation(wrow_sb[:], av_sb[0:1, :], AF.Exp, scale=0.8)
    wb_sb = const_pool.tile([P, s], bf16, name="wb_sb")
    for b in range(s // 512):
        wb_ps = tp_psum.tile([P, 512], f32, name="wb_ps", tag="tp")
        nc.tensor.matmul(wb_ps[:], ones_sb[:], wrow_sb[:, b * 512:(b + 1) * 512],
                         start=True, stop=True)
        nc.scalar.activation(wb_sb[:, b * 512:(b + 1) * 512], wb_ps[:], AF.Copy)

    # ---- Phase 3: gathered a_n -> m = e^{-0.8 a_n}, ean = e^{a_n} --------
    anf_raw = ph1_pool.tile([jc_n, P], f32, name="anf_raw")
    nc.sync.dma_start(anf_raw[:], anfull_dram[:].rearrange(
        "(k p) o -> k (p o)", p=P))
    anf_ps = tp_psum.tile([P, jc_n], f32, name="anf_ps", tag="tp")
    nc.tensor.matmul(anf_ps[:], anf_raw[:], ident[:jc_n, :jc_n],
                     is_transpose=True, start=True, stop=True)
    m_sb = const_pool.tile([P, jc_n], f32, name="m_sb")
    nc.scalar.activation(m_sb[:], anf_ps[:], AF.Exp, scale=-0.8)
    ean_sb = const_pool.tile([P, jc_n], bf16, name="ean_sb")
    nc.scalar.activation(ean_sb[:], anf_ps[:], AF.Exp, scale=1.0)

    # ---- Phase 4: gathered h2 as matmul stationaries (per-block tiles) ---
    h2blk = []
    for c in range(NCORES):
        hb = ph1_pool.tile([P, sc_n * dout], bf16, name="h2blk", tag=f"h2blk{c}")
        nc.sync.dma_start(
            hb[:],
            h2full_dram[c * s:(c + 1) * s, :].rearrange(
                "(p kl) d -> p (kl d)", kl=sc_n),
        )
        h2blk.append(hb)

    # ---- Phase 5: main loop over adj chunks ------------------------------
    nb = 512
    ib_n = s // nb
    mm_ps = [acc_psum.tile([P, nb], f32, name=f"mm_ps{b}") for b in range(ib_n)]
    rs_ps = [acc_psum.tile([1, nb], f32, name=f"rs_ps{b}") for b in range(ib_n)]
    for g in range(g_n):
        for r in range(GP):
            j = g * GP + r
            q_t = q_pool.tile([P, s], bf16, name="q_t")
            nc.vector.scalar_tensor_tensor(
                q_t[:], wb_sb[:], m_sb[:, j:j + 1],
                adj_t[g][:, r * s:(r + 1) * s],
                op0=ALU.max, op1=ALU.mult,
            )
            st = h2blk[j // sc_n][:, (j % sc_n) * dout:(j % sc_n + 1) * dout]
            for b in range(ib_n):
                nc.tensor.matmul(
                    mm_ps[b][:], st, q_t[:, b * nb:(b + 1) * nb],
                    start=(j == 0), stop=(j == jc_n - 1),
                )
            for b in range(ib_n):
                nc.tensor.matmul(
                    rs_ps[b][:], ean_sb[:, j:j + 1], q_t[:, b * nb:(b + 1) * nb],
                    start=(j == 0), stop=(j == jc_n - 1),
                )

    # ---- Phase 6: normalize, relu, transpose out -------------------------
    rs_sb = ph1_pool.tile([1, s], f32, name="rs_sb")
    for b in range(ib_n):
        nc.scalar.activation(rs_sb[:, b * nb:(b + 1) * nb], rs_ps[b][:], AF.Copy)
    rs_dram = dram_pool.tile([sc_n, P], f32, name="rs_dram")
    nc.sync.dma_start(rs_dram[:].rearrange("k p -> (k p)")[None, :], rs_sb[0:1, :])
    rs_raw = ph1_pool.tile([sc_n, P], f32, name="rs_raw")
    nc.sync.dma_start(rs_raw[:], rs_dram[:])
    rsT_ps = tp_psum.tile([P, sc_n], f32, name="rsT_ps", tag="tp")
    nc.tensor.matmul(rsT_ps[:], rs_raw[:], ident[:sc_n, :sc_n],
                     is_transpose=True, start=True, stop=True)
    rrT_sb = ph1_pool.tile([P, sc_n], f32, name="rrT_sb")
    nc.vector.reciprocal(rrT_sb[:], rsT_ps[:])

    mo_sb = ph1_pool.tile([P, s], f32, name="mo_sb")
    for b in range(ib_n):
        nc.scalar.activation(mo_sb[:, b * nb:(b + 1) * nb], mm_ps[b][:], AF.Copy)
    for c in range(sc_n):
        ot_ps = tp_psum.tile([P, P], f32, name="ot_ps", tag="tp")
        nc.tensor.matmul(
            ot_ps[:], mo_sb[:, c * P:(c + 1) * P], ident[:],
            is_transpose=True, start=True, stop=True,
        )
        oc_sb = fin_pool.tile([P, dout], f32, name="oc_sb")
        nc.scalar.activation(oc_sb[:], ot_ps[:], AF.Relu,
                             scale=rrT_sb[:, c:c + 1])
        nc.sync.dma_start(out[c * P:(c + 1) * P, :], oc_sb[:])


def build_nc(n=N, s=S, din=DIN, dout=DOUT):
    from contextlib import ExitStack

    import concourse.bacc as bacc
    import concourse.tile as tile

    nc = bacc.Bacc(
        "TRN2",
        target_bir_lowering=False,
        debug=False,
        num_devices=NCORES,
    )
    with tile.TileContext(nc) as tc, ExitStack() as ctx:
        _emit(nc, tc, ctx, n, s, din, dout)
    nc.compile()
    return nc


def prep_adjt(adj_slab):
    """[s, n] adj row-slab -> transposed [n, s] bf16 with GP-row interleave."""
    import ml_dtypes

    adjt = adj_slab.T  # [n, s]
    n, s = adjt.shape
    P = 128
    g = n // (GP * P)
    adjt = adjt.reshape(g, GP, P, s).transpose(0, 2, 1, 3).reshape(n, s)
    return np.ascontiguousarray(adjt.astype(ml_dtypes.bfloat16))


def make_in_maps(x, adj, W, attn_self, attn_neigh, s=S):
    import ml_dtypes

    bf = ml_dtypes.bfloat16
    att = np.concatenate([attn_self, attn_neigh], axis=1).astype(bf)
    wmat = np.ascontiguousarray(W.astype(bf))
    wt = np.ascontiguousarray(W.T.astype(bf))
    in_maps = []
    for c in range(NCORES):
        sl = slice(c * s, (c + 1) * s)
        in_maps.append({
            "adjt": prep_adjt(adj[sl, :]),
            "xt": np.ascontiguousarray(x[sl, :].T.astype(bf)),
            "wmat": wmat,
            "wt": wt,
            "att": att,
        })
    return in_maps


def kernel(x, adj, W, attn_self, attn_neigh):
    from concourse.bass_utils import run_bass_kernel_spmd

    x = np.asarray(x, dtype=np.float32)
    adj = np.asarray(adj, dtype=np.float32)
    W = np.asarray(W, dtype=np.float32)
    attn_self = np.asarray(attn_self, dtype=np.float32)
    attn_neigh = np.asarray(attn_neigh, dtype=np.float32)

    nc = build_nc()
    in_maps = make_in_maps(x, adj, W, attn_self, attn_neigh)
    res = run_bass_kernel_spmd(nc, in_maps, list(range(NCORES)))
    return np.concatenate([res.results[c]["out"] for c in range(NCORES)], axis=0)
